# revision 7
# baseline (speedup 1.0000x reference)
"""Dot-product attention (B=32, S=2048, D=64, per-batch key masking) on 8 trn2 cores.

Strategy: batch-shard (4 batches/core). Per batch, compute transposed scores
S^T[k, q] = (K @ Q^T) so the key-mask is a per-partition bias folded into the
ScalarE exp (activation computes exp(scale*x + bias), scale=1/8, bias=0/-1e6).
exp output (bf16) feeds matmul2 with V as the stationary operand augmented
with a ones column -> outT[65, q] where row 64 is the softmax denominator.
Final PE transpose back to [q, 65], per-partition reciprocal + scale -> out.
"""

import os
import sys

import numpy as np

_TRN_REPO = "/opt/trn_rl_repo"
if _TRN_REPO not in sys.path:
    sys.path.insert(0, _TRN_REPO)

B, S, D = 32, 2048, 64
N_CORES = 8
BPC = B // N_CORES  # batches per core
NT = S // 128  # 16 row-tiles per batch
NC_ = S // 128  # 16 key chunks
NEG = -1000000.0

_CACHE = {}


def _build_nc():
    import concourse.bacc as bacc
    import concourse.bass as bass
    import concourse.mybir as mybir
    import concourse.tile as tile

    f32 = mybir.dt.float32
    bf16 = mybir.dt.bfloat16
    Exp = mybir.ActivationFunctionType.Exp

    nc = bacc.Bacc()

    q_d = nc.dram_tensor("queries", [BPC, S, D], f32, kind="ExternalInput")
    k_d = nc.dram_tensor("keys", [BPC, S, D], f32, kind="ExternalInput")
    v_d = nc.dram_tensor("values", [BPC, S, D], f32, kind="ExternalInput")
    bias_d = nc.dram_tensor("bias", [BPC, 128, NC_], f32, kind="ExternalInput")
    out_d = nc.dram_tensor("out", [BPC, S, D], f32, kind="ExternalOutput")

    eye32 = nc.inline_tensor(np.eye(128, dtype=np.float32), name="eye32")

    with tile.TileContext(nc) as tc:
        with (
            tc.tile_pool(name="const", bufs=1) as constp,
            tc.tile_pool(name="stage", bufs=2) as stagep,
            tc.tile_pool(name="bfp", bufs=2) as bfp,
            tc.tile_pool(name="tpose", bufs=2) as tposep,
            tc.tile_pool(name="expp", bufs=6) as expp,
            tc.tile_pool(name="fin", bufs=3) as finp,
            tc.tile_pool(name="dstage", bufs=2, space="DRAM") as dstagep,
            tc.tile_pool(name="psc", bufs=2, space="PSUM") as psc,
            tc.tile_pool(name="pso", bufs=4, space="PSUM") as pso,
        ):
            id32r = constp.tile([128, 128], f32, name="id32r")
            nc.sync.dma_start(id32r[:], eye32[:])
            id32 = constp.tile([128, 128], f32, name="id32")
            nc.vector.tensor_copy(id32[:], id32r[:])

            pending = []

            def late_finalize(item):
                fb, osb = item
                outsb = finp.tile([128, NT * D], f32, name="outsb", tag="outsb")
                for t in range(NT):
                    tf = pso.tile([128, 65], f32, name="tf", tag="oT")
                    nc.tensor.transpose(
                        tf[:], osb[:, 128 * t : 128 * (t + 1)], id32[0:65, 0:65]
                    )
                    rc = constp.tile([128, 1], f32, name="rc", tag="rc", bufs=4)
                    nc.vector.reciprocal(rc[:], tf[:, 64:65])
                    nc.vector.tensor_scalar_mul(
                        outsb[:, D * t : D * (t + 1)], tf[:, 0:D], rc[:]
                    )
                nc.sync.dma_start(
                    out_d[fb].rearrange("(t p) d -> p t d", p=128),
                    outsb.rearrange("p (t d) -> p t d", d=D),
                )

            for b in range(BPC):
                # ---- load + cast ----
                qs = stagep.tile([128, NT * D], f32, name="qs", tag="qs")
                nc.sync.dma_start(qs.rearrange("p (t d) -> p t d", d=D), q_d[b].rearrange("(t p) d -> p t d", p=128))
                ks = stagep.tile([128, NT * D], f32, name="ks", tag="ks")
                nc.sync.dma_start(ks.rearrange("p (t d) -> p t d", d=D), k_d[b].rearrange("(t p) d -> p t d", p=128))
                vs = stagep.tile([128, NT * D], f32, name="vs", tag="vs")
                nc.sync.dma_start(vs.rearrange("p (t d) -> p t d", d=D), v_d[b].rearrange("(t p) d -> p t d", p=128))
                bias_t = constp.tile([128, NC_], f32, name="bias_t", tag="bias", bufs=4)
                nc.sync.dma_start(bias_t[:], bias_d[b][:])

                qb = bfp.tile([128, NT * D], bf16, name="qb", tag="qb")
                nc.vector.tensor_copy(qb[:], qs[:])
                kb = bfp.tile([128, NT * D], bf16, name="kb", tag="kb")
                nc.vector.tensor_copy(kb[:], ks[:])
                # V' with ones column: [128, 16, 65]
                vt = bfp.tile([128, NT * (D + 1)], bf16, name="vt", tag="vt")
                vt3 = vt.rearrange("p (c w) -> p c w", w=D + 1)
                nc.vector.tensor_copy(
                    vt3[:, :, 0:D], vs.rearrange("p (c d) -> p c d", d=D)
                )
                nc.vector.memset(vt3[:, :, D : D + 1], 1.0)

                # ---- transposes via DMA xbar: qkT [128, 2048] = (Q|K).T ----
                qkst = dstagep.tile([S, 128], bf16, name="qkst", tag="qkst")
                qkst3 = qkst.rearrange("(t p) c -> p t c", p=128)
                nc.sync.dma_start(qkst3[:, :, 0:D], qb.rearrange("p (t d) -> p t d", d=D))
                nc.sync.dma_start(qkst3[:, :, D : 2 * D], kb.rearrange("p (t d) -> p t d", d=D))
                qkT = tposep.tile([128, S], bf16, name="qkT", tag="qkT")
                nc.sync.dma_start_transpose(qkT[:], qkst[:])
                qt = qkT[0:64, :]
                kt = tposep.tile([64, S], bf16, name="kt", tag="kt")
                nc.vector.tensor_copy(kt[:], qkT[64:128, :])

                # ---- main loop over key chunks ----
                oT = [
                    pso.tile([65, 512], f32, name=f"oT{j}", tag="oT") for j in range(4)
                ]
                for c in range(NC_):
                    for h in range(2):
                        sc = psc.tile([128, 1024], f32, name="sc", tag="scores")
                        for jj in range(2):
                            nc.tensor.matmul(
                                sc[:, 512 * jj : 512 * (jj + 1)],
                                kt[:, 128 * c : 128 * (c + 1)],
                                qt[:, 1024 * h + 512 * jj : 1024 * h + 512 * (jj + 1)],
                                start=True,
                                stop=True,
                            )
                        ex = expp.tile([128, 1024], bf16, name="ex", tag="ex")
                        nc.scalar.activation(
                            ex[:], sc[:], Exp, bias=bias_t[:, c : c + 1], scale=0.125
                        )
                        for jj in range(2):
                            nc.tensor.matmul(
                                oT[2 * h + jj][:],
                                vt3[:, c, :],
                                ex[:, 512 * jj : 512 * (jj + 1)],
                                start=(c == 0),
                                stop=(c == NC_ - 1),
                            )

                # ---- finalize part 1: outT -> SBUF (frees oT slots) ----
                osb = finp.tile([65, S], f32, name="osb", tag="osb")
                for j in range(4):
                    nc.vector.tensor_copy(osb[:, 512 * j : 512 * (j + 1)], oT[j][:])
                pending.append((b, osb))
                # deferred finalize of the previous batch overlaps this batch's tail
                if b > 0:
                    late_finalize(pending.pop(0))

            late_finalize(pending.pop(0))

    nc.compile()
    return nc


def _get_nc():
    if "nc" not in _CACHE:
        _CACHE["nc"] = _build_nc()
    return _CACHE["nc"]


def run_on_device(in_maps, trace=False):
    from concourse.bass_utils import run_bass_kernel_spmd

    nc = _get_nc()
    return run_bass_kernel_spmd(
        nc, in_maps, core_ids=list(range(N_CORES)), trace=trace
    )


def make_in_maps(queries, keys, values, valid_lens):
    queries = np.ascontiguousarray(np.asarray(queries, dtype=np.float32))
    keys = np.ascontiguousarray(np.asarray(keys, dtype=np.float32))
    values = np.ascontiguousarray(np.asarray(values, dtype=np.float32))
    valid_lens = np.asarray(valid_lens, dtype=np.int32)

    # bias[b, p, c] = 0 if key index c*128+p < valid_len else NEG
    kidx = (np.arange(NC_)[None, :] * 128 + np.arange(128)[:, None])[None]  # [1,128,16]
    bias = np.where(kidx < valid_lens[:, None, None], 0.0, NEG).astype(np.float32)

    in_maps = []
    for c in range(N_CORES):
        sl = slice(c * BPC, (c + 1) * BPC)
        in_maps.append(
            {
                "queries": queries[sl],
                "keys": keys[sl],
                "values": values[sl],
                "bias": np.ascontiguousarray(bias[sl]),
            }
        )
    return in_maps


def kernel(**inputs):
    in_maps = make_in_maps(
        inputs["queries"], inputs["keys"], inputs["values"], inputs["valid_lens"]
    )
    res = run_on_device(in_maps, trace=False)
    return np.concatenate([r["out"] for r in res.results], axis=0)


if __name__ == "__main__":
    _build_nc()
    print("build OK")
